# revision 54
# baseline (speedup 1.0000x reference)
"""Dynamic-kernel CNN (conv5x5->tanh gate->windowed sum) on 8 trn2 cores.

out(y,x) = sum_{dx,dy} xq[y+dy, x+dx] * tanh( sum_{k} W2[c,k] V_k + b_c ),
with xq = pad2(x) [32x32], c = k = 5*dx+dy, V_k(y,x) = xq[y+dy, x+dx].

Data-parallel over batch: 2048 images -> 256 per core.

Per-core layout: partitions = (strip s in 0..4) x (tap k) = 125, with
q = 25*s + 5*dy + dx.  Free dim = pixel plane (28*28 = 784).
A chunk = up to 4 consecutive groups of 5 images (20 images).

bf16 datapath: matmuls run 1 cycle/row (fp32 is 4), gathers move half
the bytes.  The V gather is two-stage because DMA in-APs cap at 3 dims:
  stage A (per group):  V900[(s,dy), g*904+p] = xq[strip s][32*dy+p]
     (linearizes (s,dy) onto 25 partitions; [[P,5],[32,5],[1,904]])
  stage B (per chunk):  V25[(s,dy,dx), g*904+p] = V900[(s,dy)][g*904+dx+p]
     (the x25 dx-replication; [[pitch,25],[1,5],[1,904*ncg]])
Each dma_start holds its issuing engine's sequencer for the transfer,
so gathers alternate between sync (HWDGE, 5 shared DMA engines) and
gpsimd (SWDGE, all 16 engines); bulk input loads ride the scalar ring
and are emitted lazily so their transfers don't crowd the startup;
round 0 ramps chunk sizes (1,2,4..) and sends its first B-transfers to
gpsimd so the first FC starts ~15us earlier.  NOTE: the chip runs
power-throttled here (util limit ~0.5, active ~60% of the time), so
per-op times are ~2x the unthrottled model.

Pipeline per group of 5 images:
  2. FC = blockdiag(W2^T)^T @ V   (two bf16 matmuls, one 2-bank PSUM tile)
  3. G = tanh(FC + b) on ACT      (one strided ACTIVATE, bias fused)
  4. M = V * G                    (DVE, bf16)
  5. per-strip channel reduce: bf16 matmul with a zero-padded ones
     lhsT slice placing group j at partitions 5j..5j+4, accumulating
     25 groups into a round-level PSUM pair [125, 392]x2.
  6. per round: evacuate PSUM -> SBUF fp32 -> 2 DMAs to y rows.
"""

import numpy as np
from contextlib import ExitStack

import concourse.bass as bass
import concourse.tile as tile
from concourse import bacc, mybir
from concourse import bass_utils

F32 = mybir.dt.float32
BF16 = mybir.dt.bfloat16
TANH = mybir.ActivationFunctionType.Tanh

N_CORES = 8
B_FULL = 2048
B_LOC = B_FULL // N_CORES  # 256
NPIX = 784                 # 28*28
XQ_LEN = 1024              # 32*32 padded plane
VROW = 904                 # per-group V row pitch: 28*32 window + dy/dx slack
HALF = 392                 # half pixel plane
NC_MAX = 4                 # groups per stage-B chunk

# image -> (round r, group j, strip s): img = 125*r + 5*j + s
ROUNDS = ((0, 25, 125), (1, 25, 125), (2, 2, 6))  # (r, n_groups, rows stored)


def _emit(ctx, tc, x_d, wblk_d, ones5_d, bias_d, y_d):
    nc = tc.nc

    cpool = ctx.enter_context(tc.tile_pool(name="const", bufs=1))
    apool = ctx.enter_context(tc.tile_pool(name="v900", bufs=4))
    vpool = ctx.enter_context(tc.tile_pool(name="v25", bufs=4))
    gpool = ctx.enter_context(tc.tile_pool(name="g", bufs=4))
    mpool = ctx.enter_context(tc.tile_pool(name="m", bufs=6))
    epool = ctx.enter_context(tc.tile_pool(name="evac", bufs=3))
    pfc = ctx.enter_context(tc.tile_pool(name="pfc", bufs=2, space="PSUM"))
    pred = ctx.enter_context(tc.tile_pool(name="pred", bufs=2, space="PSUM"))

    # consts ride the scalar queue: sync/gpsimd must start gathering ASAP
    wblk = cpool.tile([125, 125], BF16)
    nc.scalar.dma_start(wblk[:], wblk_d[:])
    mbig = cpool.tile([125, 245], BF16)
    nc.scalar.dma_start(mbig[:], ones5_d[:])
    biasv = cpool.tile([125, 1], F32)
    nc.scalar.dma_start(biasv[:], bias_d[:])

    # padded bf16 images: partition p holds xq of image 125*r + p at cols
    # r*1024. 32 extra tail cols: stage-A reads run to 1024*r + 1032.
    xq = cpool.tile([128, 3 * XQ_LEN + 32], BF16)
    stage = cpool.tile([128, 3 * NPIX], F32)
    # per-round memsets so round 0's cast isn't gated on zeroing it all
    nc.vector.memset(xq[:, 0 : XQ_LEN + 32], 0.0)
    nc.vector.memset(xq[:, XQ_LEN + 32 : 2 * XQ_LEN + 32], 0.0)
    nc.vector.memset(xq[:, 2 * XQ_LEN + 32 :], 0.0)
    # bulk loads ride the scalar ring AND are emitted lazily mid-loop so
    # their transfers don't compete with the first chunks' gathers for
    # the shared DMA engines; only round 0's head loads up front.
    def emit_load(r, lo, hi, leng):
        leng.dma_start(
            stage[lo:hi, NPIX * r : NPIX * (r + 1)],
            x_d[125 * r + lo : 125 * r + hi, :],
        )
        src = stage[lo:hi, NPIX * r : NPIX * (r + 1)].rearrange(
            "p (y x) -> p y x", x=28
        )
        dst = xq[lo:hi, XQ_LEN * r : XQ_LEN * (r + 1)].rearrange(
            "p (y x) -> p y x", x=32
        )[:, 2:30, 2:30]
        nc.vector.tensor_copy(dst, src)

    emit_load(0, 0, 64, nc.sync)
    # emitted after the Nth global chunk: (N, args)
    deferred_loads = [
        (1, (0, 64, 125, nc.scalar)),
        (3, (1, 0, 125, nc.scalar)),
        (6, (2, 0, 6, nc.scalar)),
    ]

    xq_ap = xq[:]
    xq_pitch = xq_ap.ap[0][0]  # partition stride in elements

    # gather/output DMAs alternate between sync and gpsimd; scalar is
    # saturated by the tanh ACTIVATEs.
    issuers = [nc.sync, nc.gpsimd]
    chunk_idx = 0

    for r, n_groups, rows in ROUNDS:
        red_a = pred.tile([125, HALF], F32, tag="red_a")
        red_b = pred.tile([125, HALF], F32, tag="red_b")
        # round 0 ramps up chunk size so the first FC starts ASAP
        if r == 0:
            sizes = [1, 2] + [NC_MAX] * ((n_groups - 3) // NC_MAX)
            sizes += [n_groups - sum(sizes)] if sum(sizes) < n_groups else []
        else:
            sizes = [NC_MAX] * (n_groups // NC_MAX)
            sizes += [n_groups - sum(sizes)] if sum(sizes) < n_groups else []
        j0 = 0
        for ncg in sizes:
            # all B's ride gpsimd's 16-engine queue (the 5 shared HWDGE
            # engines are too slow for the big transfers); the alternating
            # A's keep a producer on B's own queue for safe ordering
            eng = nc.gpsimd
            chunk_idx += 1
            while deferred_loads and chunk_idx > deferred_loads[0][0]:
                emit_load(*deferred_loads.pop(0)[1])

            # --- 1a. stage A: V900g [25=(s,dy), ncg*904+8] per group ---
            # the chunk's A-DMAs run on BOTH issuers in parallel
            v900 = apool.tile([25, NC_MAX * VROW + 8], BF16)
            v900_ap = v900[:]
            v900_pitch = v900_ap.ap[0][0]
            for g in range(ncg):
                in_a = bass.AP(
                    tensor=xq_ap.tensor,
                    offset=xq_pitch * 5 * (j0 + g) + XQ_LEN * r,
                    ap=[[xq_pitch, 5], [32, 5], [1, VROW]],
                )
                issuers[g % 2].dma_start(
                    v900[:, VROW * g : VROW * (g + 1)], in_a
                )

            # --- 1b. stage B: V25 [125=(s,dy,dx), ncg*904] in one DMA ---
            # (partition-stride dim must be the first AP dim)
            v = vpool.tile([125, NC_MAX * VROW + 8], BF16)
            in_b = bass.AP(
                tensor=v900_ap.tensor,
                offset=v900_ap.offset,
                ap=[[v900_pitch, 25], [1, 5], [1, VROW * ncg]],
            )
            eng.dma_start(v[:, 0 : VROW * ncg], in_b)

            for g in range(ncg):
                # strided views of the real 28x28 pixel plane
                vyx = (
                    v[:, VROW * g : VROW * g + 896]
                    .rearrange("p (y xc) -> p y xc", xc=32)[:, :, 0:28]
                )

                # --- 2. FC matmuls into one 2-bank PSUM tile ---
                fc = pfc.tile([125, 1024], F32)
                nc.tensor.matmul(
                    fc[:, 0:HALF], wblk[:], vyx[:, 0:14],
                    start=True, stop=True,
                )
                nc.tensor.matmul(
                    fc[:, 512 : 512 + HALF], wblk[:], vyx[:, 14:28],
                    start=True, stop=True,
                )

                # --- 3. G = tanh(FC + b), one strided ACT over both banks ---
                g_t = gpool.tile([125, NPIX], BF16)
                fcv = fc[:].rearrange("p (t c) -> p t c", c=512)[:, :, 0:HALF]
                gv = g_t[:].rearrange("p (t c) -> p t c", c=HALF)
                nc.scalar.activation(gv, fcv, TANH, bias=biasv[:], scale=1.0)

                # --- 4. M = V * G (DVE) ---
                m = mpool.tile([125, NPIX], BF16)
                gyx = g_t[:].rearrange("p (y x) -> p y x", x=28)
                myx = m[:].rearrange("p (y x) -> p y x", x=28)
                nc.vector.tensor_mul(myx, vyx, gyx)

                # --- 5. per-strip channel reduce, placed at partitions 5j.. ---
                j = j0 + g
                ones_j = mbig[:, 120 - 5 * j : 245 - 5 * j]
                nc.tensor.matmul(
                    red_a[:], ones_j, m[:, 0:HALF],
                    start=(j == 0), stop=(j == n_groups - 1),
                    skip_group_check=True,
                )
                nc.tensor.matmul(
                    red_b[:], ones_j, m[:, HALF:NPIX],
                    start=(j == 0), stop=(j == n_groups - 1),
                    skip_group_check=True,
                )
            j0 += ncg

        # --- 6. evacuate + store round ---
        e_a = epool.tile([125, HALF], F32, tag="e_a")
        nc.vector.tensor_copy(e_a[:], red_a[:])
        nc.sync.dma_start(y_d[125 * r : 125 * r + rows, 0:HALF], e_a[0:rows, :])
        e_b = epool.tile([125, HALF], F32, tag="e_b")
        nc.vector.tensor_copy(e_b[:], red_b[:])
        nc.gpsimd.dma_start(
            y_d[125 * r : 125 * r + rows, HALF:NPIX], e_b[0:rows, :]
        )


def build():
    nc = bacc.Bacc("TRN2", target_bir_lowering=False, debug=False)
    x_d = nc.dram_tensor("x", [B_LOC, NPIX], F32, kind="ExternalInput").ap()
    wblk_d = nc.dram_tensor("wblk", [125, 125], BF16, kind="ExternalInput").ap()
    ones5_d = nc.dram_tensor("mbig", [125, 245], BF16, kind="ExternalInput").ap()
    bias_d = nc.dram_tensor("biasv", [125, 1], F32, kind="ExternalInput").ap()
    y_d = nc.dram_tensor("y", [B_LOC, NPIX], F32, kind="ExternalOutput").ap()

    with tile.TileContext(nc) as tc:
        with ExitStack() as ctx:
            _emit(ctx, tc, x_d, wblk_d, ones5_d, bias_d, y_d)
    nc.compile()
    return nc


def make_consts(W, b):
    import ml_dtypes

    W = np.asarray(W, dtype=np.float32)
    b = np.asarray(b, dtype=np.float32)
    # W2[c, 5*dx+dy] = W[c, 0, dy, dx]
    W2 = W[:, 0].transpose(0, 2, 1).reshape(25, 25)
    wblk = np.zeros((125, 125), dtype=np.float32)
    for s in range(5):
        wblk[25 * s : 25 * s + 25, 25 * s : 25 * s + 25] = W2.T
    mbig = np.zeros((125, 245), dtype=np.float32)
    for s in range(5):
        mbig[25 * s : 25 * s + 25, 120 + s] = 1.0
    biasv = np.tile(b, 5).astype(np.float32)[:, None]
    # permute from the (s, k) layout to the gather's q = (s, dy, dx) layout
    perm = np.zeros(125, dtype=np.int64)
    for s in range(5):
        for dy in range(5):
            for dx in range(5):
                perm[25 * s + 5 * dy + dx] = 25 * s + 5 * dx + dy
    wblk = wblk[perm][:, perm]
    mbig = mbig[perm]
    biasv = biasv[perm]
    wblk = wblk.astype(ml_dtypes.bfloat16)
    mbig = mbig.astype(ml_dtypes.bfloat16)
    return wblk, mbig, biasv


_NC_CACHE = None


def get_nc():
    global _NC_CACHE
    if _NC_CACHE is None:
        _NC_CACHE = build()
    return _NC_CACHE


def run(x, W, b, **spmd_kwargs):
    x = np.ascontiguousarray(np.asarray(x, dtype=np.float32))
    wblk, mbig, biasv = make_consts(W, b)
    xs = x.reshape(N_CORES, B_LOC, NPIX)
    in_maps = [
        {"x": xs[c], "wblk": wblk, "mbig": mbig, "biasv": biasv}
        for c in range(N_CORES)
    ]
    nc = get_nc()
    res = bass_utils.run_bass_kernel_spmd(
        nc, in_maps, list(range(N_CORES)), **spmd_kwargs
    )
    y = np.concatenate([res.results[c]["y"] for c in range(N_CORES)], axis=0)
    return y.reshape(B_FULL, 1, 28, 28), res


def kernel(x, W, b):
    y, _ = run(x, W, b)
    return y.astype(np.float32)


# revision 55
# speedup vs baseline: 1.0191x; 1.0191x over previous
"""Dynamic-kernel CNN (conv5x5->tanh gate->windowed sum) on 8 trn2 cores.

out(y,x) = sum_{dx,dy} xq[y+dy, x+dx] * tanh( sum_{k} W2[c,k] V_k + b_c ),
with xq = pad2(x) [32x32], c = k = 5*dx+dy, V_k(y,x) = xq[y+dy, x+dx].

Data-parallel over batch: 2048 images -> 256 per core.

Per-core layout: partitions = (strip s in 0..4) x (tap k) = 125, with
q = 25*s + 5*dy + dx.  Free dim = pixel plane (28*28 = 784).
A chunk = up to 4 consecutive groups of 5 images (20 images).

bf16 datapath: matmuls run 1 cycle/row (fp32 is 4), gathers move half
the bytes.  The V gather is two-stage because DMA in-APs cap at 3 dims:
  stage A (per group):  V900[(s,dy), g*904+p] = xq[strip s][32*dy+p]
     (linearizes (s,dy) onto 25 partitions; [[P,5],[32,5],[1,904]])
  stage B (per chunk):  V25[(s,dy,dx), g*904+p] = V900[(s,dy)][g*904+dx+p]
     (the x25 dx-replication; [[pitch,25],[1,5],[1,904*ncg]])
Each dma_start holds its issuing engine's sequencer for the transfer,
so gathers alternate between sync (HWDGE, 5 shared DMA engines) and
gpsimd (SWDGE, all 16 engines); bulk input loads ride the scalar ring
and are emitted lazily so their transfers don't crowd the startup;
round 0 ramps chunk sizes (1,2,4..) and sends its first B-transfers to
gpsimd so the first FC starts ~15us earlier.  NOTE: the chip runs
power-throttled here (util limit ~0.5, active ~60% of the time), so
per-op times are ~2x the unthrottled model.

Pipeline per group of 5 images:
  2. FC = blockdiag(W2^T)^T @ V   (two bf16 matmuls, one 2-bank PSUM tile)
  3. G = tanh(FC + b) on ACT      (one strided ACTIVATE, bias fused)
  4. M = V * G                    (DVE, bf16)
  5. per-strip channel reduce: bf16 matmul with a zero-padded ones
     lhsT slice placing group j at partitions 5j..5j+4, accumulating
     25 groups into a round-level PSUM pair [125, 392]x2.
  6. per round: evacuate PSUM -> SBUF fp32 -> 2 DMAs to y rows.
"""

import numpy as np
from contextlib import ExitStack

import concourse.bass as bass
import concourse.tile as tile
from concourse import bacc, mybir
from concourse import bass_utils

F32 = mybir.dt.float32
BF16 = mybir.dt.bfloat16
TANH = mybir.ActivationFunctionType.Tanh

N_CORES = 8
B_FULL = 2048
B_LOC = B_FULL // N_CORES  # 256
NPIX = 784                 # 28*28
XQ_LEN = 1024              # 32*32 padded plane
VROW = 904                 # per-group V row pitch: 28*32 window + dy/dx slack
HALF = 392                 # half pixel plane
NC_MAX = 4                 # groups per stage-B chunk

# image -> (round r, group j, strip s): img = 125*r + 5*j + s
ROUNDS = ((0, 25, 125), (1, 25, 125), (2, 2, 6))  # (r, n_groups, rows stored)


def _emit(ctx, tc, x_d, wblk_d, ones5_d, bias_d, y_d):
    nc = tc.nc

    cpool = ctx.enter_context(tc.tile_pool(name="const", bufs=1))
    apool = ctx.enter_context(tc.tile_pool(name="v900", bufs=6))
    vpool = ctx.enter_context(tc.tile_pool(name="v25", bufs=6))
    gpool = ctx.enter_context(tc.tile_pool(name="g", bufs=6))
    mpool = ctx.enter_context(tc.tile_pool(name="m", bufs=8))
    epool = ctx.enter_context(tc.tile_pool(name="evac", bufs=3))
    pfc = ctx.enter_context(tc.tile_pool(name="pfc", bufs=2, space="PSUM"))
    pred = ctx.enter_context(tc.tile_pool(name="pred", bufs=2, space="PSUM"))

    # consts ride the scalar queue: sync/gpsimd must start gathering ASAP
    wblk = cpool.tile([125, 125], BF16)
    nc.scalar.dma_start(wblk[:], wblk_d[:])
    mbig = cpool.tile([125, 245], BF16)
    nc.scalar.dma_start(mbig[:], ones5_d[:])
    biasv = cpool.tile([125, 1], F32)
    nc.scalar.dma_start(biasv[:], bias_d[:])

    # padded bf16 images: partition p holds xq of image 125*r + p at cols
    # r*1024. 32 extra tail cols: stage-A reads run to 1024*r + 1032.
    xq = cpool.tile([128, 3 * XQ_LEN + 32], BF16)
    stage = cpool.tile([128, 3 * NPIX], F32)
    # per-round memsets so round 0's cast isn't gated on zeroing it all
    nc.vector.memset(xq[:, 0 : XQ_LEN + 32], 0.0)
    nc.vector.memset(xq[:, XQ_LEN + 32 : 2 * XQ_LEN + 32], 0.0)
    nc.vector.memset(xq[:, 2 * XQ_LEN + 32 :], 0.0)
    # bulk loads ride the scalar ring AND are emitted lazily mid-loop so
    # their transfers don't compete with the first chunks' gathers for
    # the shared DMA engines; only round 0's head loads up front.
    def emit_load(r, lo, hi, leng):
        leng.dma_start(
            stage[lo:hi, NPIX * r : NPIX * (r + 1)],
            x_d[125 * r + lo : 125 * r + hi, :],
        )
        src = stage[lo:hi, NPIX * r : NPIX * (r + 1)].rearrange(
            "p (y x) -> p y x", x=28
        )
        dst = xq[lo:hi, XQ_LEN * r : XQ_LEN * (r + 1)].rearrange(
            "p (y x) -> p y x", x=32
        )[:, 2:30, 2:30]
        nc.vector.tensor_copy(dst, src)

    emit_load(0, 0, 64, nc.sync)
    # emitted after the Nth global chunk: (N, args)
    deferred_loads = [
        (1, (0, 64, 125, nc.scalar)),
        (3, (1, 0, 125, nc.scalar)),
        (6, (2, 0, 6, nc.scalar)),
    ]

    xq_ap = xq[:]
    xq_pitch = xq_ap.ap[0][0]  # partition stride in elements

    # gather/output DMAs alternate between sync and gpsimd; scalar is
    # saturated by the tanh ACTIVATEs.
    issuers = [nc.sync, nc.gpsimd]
    chunk_idx = 0

    for r, n_groups, rows in ROUNDS:
        red_a = pred.tile([125, HALF], F32, tag="red_a")
        red_b = pred.tile([125, HALF], F32, tag="red_b")
        # round 0 ramps up chunk size so the first FC starts ASAP
        if r == 0:
            sizes = [1, 2] + [NC_MAX] * ((n_groups - 3) // NC_MAX)
            sizes += [n_groups - sum(sizes)] if sum(sizes) < n_groups else []
        else:
            sizes = [NC_MAX] * (n_groups // NC_MAX)
            sizes += [n_groups - sum(sizes)] if sum(sizes) < n_groups else []
        j0 = 0
        for ncg in sizes:
            # first chunks' B rides gpsimd's 16-engine queue to beat the
            # startup contention on the 5 shared HWDGE engines
            eng = nc.gpsimd if chunk_idx < 4 else issuers[chunk_idx % 2]
            chunk_idx += 1
            while deferred_loads and chunk_idx > deferred_loads[0][0]:
                emit_load(*deferred_loads.pop(0)[1])

            # --- 1a. stage A: V900g [25=(s,dy), ncg*904+8] per group ---
            # the chunk's A-DMAs run on BOTH issuers in parallel
            v900 = apool.tile([25, NC_MAX * VROW + 8], BF16)
            v900_ap = v900[:]
            v900_pitch = v900_ap.ap[0][0]
            for g in range(ncg):
                in_a = bass.AP(
                    tensor=xq_ap.tensor,
                    offset=xq_pitch * 5 * (j0 + g) + XQ_LEN * r,
                    ap=[[xq_pitch, 5], [32, 5], [1, VROW]],
                )
                issuers[g % 2].dma_start(
                    v900[:, VROW * g : VROW * (g + 1)], in_a
                )

            # --- 1b. stage B: V25 [125=(s,dy,dx), ncg*904] in one DMA ---
            # (partition-stride dim must be the first AP dim)
            v = vpool.tile([125, NC_MAX * VROW + 8], BF16)
            in_b = bass.AP(
                tensor=v900_ap.tensor,
                offset=v900_ap.offset,
                ap=[[v900_pitch, 25], [1, 5], [1, VROW * ncg]],
            )
            eng.dma_start(v[:, 0 : VROW * ncg], in_b)

            for g in range(ncg):
                # strided views of the real 28x28 pixel plane
                vyx = (
                    v[:, VROW * g : VROW * g + 896]
                    .rearrange("p (y xc) -> p y xc", xc=32)[:, :, 0:28]
                )

                # --- 2. FC matmuls into one 2-bank PSUM tile ---
                fc = pfc.tile([125, 1024], F32)
                nc.tensor.matmul(
                    fc[:, 0:HALF], wblk[:], vyx[:, 0:14],
                    start=True, stop=True,
                )
                nc.tensor.matmul(
                    fc[:, 512 : 512 + HALF], wblk[:], vyx[:, 14:28],
                    start=True, stop=True,
                )

                # --- 3. G = tanh(FC + b), one strided ACT over both banks ---
                g_t = gpool.tile([125, NPIX], BF16)
                fcv = fc[:].rearrange("p (t c) -> p t c", c=512)[:, :, 0:HALF]
                gv = g_t[:].rearrange("p (t c) -> p t c", c=HALF)
                nc.scalar.activation(gv, fcv, TANH, bias=biasv[:], scale=1.0)

                # --- 4. M = V * G (DVE) ---
                m = mpool.tile([125, NPIX], BF16)
                gyx = g_t[:].rearrange("p (y x) -> p y x", x=28)
                myx = m[:].rearrange("p (y x) -> p y x", x=28)
                nc.vector.tensor_mul(myx, vyx, gyx)

                # --- 5. per-strip channel reduce, placed at partitions 5j.. ---
                j = j0 + g
                ones_j = mbig[:, 120 - 5 * j : 245 - 5 * j]
                nc.tensor.matmul(
                    red_a[:], ones_j, m[:, 0:HALF],
                    start=(j == 0), stop=(j == n_groups - 1),
                    skip_group_check=True,
                )
                nc.tensor.matmul(
                    red_b[:], ones_j, m[:, HALF:NPIX],
                    start=(j == 0), stop=(j == n_groups - 1),
                    skip_group_check=True,
                )
            j0 += ncg

        # --- 6. evacuate + store round ---
        e_a = epool.tile([125, HALF], F32, tag="e_a")
        nc.vector.tensor_copy(e_a[:], red_a[:])
        nc.sync.dma_start(y_d[125 * r : 125 * r + rows, 0:HALF], e_a[0:rows, :])
        e_b = epool.tile([125, HALF], F32, tag="e_b")
        nc.vector.tensor_copy(e_b[:], red_b[:])
        nc.gpsimd.dma_start(
            y_d[125 * r : 125 * r + rows, HALF:NPIX], e_b[0:rows, :]
        )


def build():
    nc = bacc.Bacc("TRN2", target_bir_lowering=False, debug=False)
    x_d = nc.dram_tensor("x", [B_LOC, NPIX], F32, kind="ExternalInput").ap()
    wblk_d = nc.dram_tensor("wblk", [125, 125], BF16, kind="ExternalInput").ap()
    ones5_d = nc.dram_tensor("mbig", [125, 245], BF16, kind="ExternalInput").ap()
    bias_d = nc.dram_tensor("biasv", [125, 1], F32, kind="ExternalInput").ap()
    y_d = nc.dram_tensor("y", [B_LOC, NPIX], F32, kind="ExternalOutput").ap()

    with tile.TileContext(nc) as tc:
        with ExitStack() as ctx:
            _emit(ctx, tc, x_d, wblk_d, ones5_d, bias_d, y_d)
    nc.compile()
    return nc


def make_consts(W, b):
    import ml_dtypes

    W = np.asarray(W, dtype=np.float32)
    b = np.asarray(b, dtype=np.float32)
    # W2[c, 5*dx+dy] = W[c, 0, dy, dx]
    W2 = W[:, 0].transpose(0, 2, 1).reshape(25, 25)
    wblk = np.zeros((125, 125), dtype=np.float32)
    for s in range(5):
        wblk[25 * s : 25 * s + 25, 25 * s : 25 * s + 25] = W2.T
    mbig = np.zeros((125, 245), dtype=np.float32)
    for s in range(5):
        mbig[25 * s : 25 * s + 25, 120 + s] = 1.0
    biasv = np.tile(b, 5).astype(np.float32)[:, None]
    # permute from the (s, k) layout to the gather's q = (s, dy, dx) layout
    perm = np.zeros(125, dtype=np.int64)
    for s in range(5):
        for dy in range(5):
            for dx in range(5):
                perm[25 * s + 5 * dy + dx] = 25 * s + 5 * dx + dy
    wblk = wblk[perm][:, perm]
    mbig = mbig[perm]
    biasv = biasv[perm]
    wblk = wblk.astype(ml_dtypes.bfloat16)
    mbig = mbig.astype(ml_dtypes.bfloat16)
    return wblk, mbig, biasv


_NC_CACHE = None


def get_nc():
    global _NC_CACHE
    if _NC_CACHE is None:
        _NC_CACHE = build()
    return _NC_CACHE


def run(x, W, b, **spmd_kwargs):
    x = np.ascontiguousarray(np.asarray(x, dtype=np.float32))
    wblk, mbig, biasv = make_consts(W, b)
    xs = x.reshape(N_CORES, B_LOC, NPIX)
    in_maps = [
        {"x": xs[c], "wblk": wblk, "mbig": mbig, "biasv": biasv}
        for c in range(N_CORES)
    ]
    nc = get_nc()
    res = bass_utils.run_bass_kernel_spmd(
        nc, in_maps, list(range(N_CORES)), **spmd_kwargs
    )
    y = np.concatenate([res.results[c]["y"] for c in range(N_CORES)], axis=0)
    return y.reshape(B_FULL, 1, 28, 28), res


def kernel(x, W, b):
    y, _ = run(x, W, b)
    return y.astype(np.float32)


# revision 56
# speedup vs baseline: 1.0271x; 1.0079x over previous
"""Dynamic-kernel CNN (conv5x5->tanh gate->windowed sum) on 8 trn2 cores.

out(y,x) = sum_{dx,dy} xq[y+dy, x+dx] * tanh( sum_{k} W2[c,k] V_k + b_c ),
with xq = pad2(x) [32x32], c = k = 5*dx+dy, V_k(y,x) = xq[y+dy, x+dx].

Data-parallel over batch: 2048 images -> 256 per core.

Per-core layout: partitions = (strip s in 0..4) x (tap k) = 125, with
q = 25*s + 5*dy + dx.  Free dim = pixel plane (28*28 = 784).
A chunk = up to 4 consecutive groups of 5 images (20 images).

bf16 datapath: matmuls run 1 cycle/row (fp32 is 4), gathers move half
the bytes.  The V gather is two-stage because DMA in-APs cap at 3 dims:
  stage A (per group):  V900[(s,dy), g*904+p] = xq[strip s][32*dy+p]
     (linearizes (s,dy) onto 25 partitions; [[P,5],[32,5],[1,904]])
  stage B (per chunk):  V25[(s,dy,dx), g*904+p] = V900[(s,dy)][g*904+dx+p]
     (the x25 dx-replication; [[pitch,25],[1,5],[1,904*ncg]])
Each dma_start holds its issuing engine's sequencer for the transfer,
so gathers alternate between sync (HWDGE, 5 shared DMA engines) and
gpsimd (SWDGE, all 16 engines); bulk input loads ride the scalar ring
and are emitted lazily so their transfers don't crowd the startup;
round 0 ramps chunk sizes (1,2,4..) and sends its first B-transfers to
gpsimd so the first FC starts ~15us earlier.  NOTE: the chip runs
power-throttled here (util limit ~0.5, active ~60% of the time), so
per-op times are ~2x the unthrottled model.

Pipeline per group of 5 images:
  2. FC = blockdiag(W2^T)^T @ V   (two bf16 matmuls, one 2-bank PSUM tile)
  3. G = tanh(FC + b) on ACT      (one strided ACTIVATE, bias fused)
  4. M = V * G                    (DVE, bf16)
  5. per-strip channel reduce: bf16 matmul with a zero-padded ones
     lhsT slice placing group j at partitions 5j..5j+4, accumulating
     25 groups into a round-level PSUM pair [125, 392]x2.
  6. per round: evacuate PSUM -> SBUF fp32 -> 2 DMAs to y rows.
"""

import numpy as np
from contextlib import ExitStack

import concourse.bass as bass
import concourse.tile as tile
from concourse import bacc, mybir
from concourse import bass_utils

F32 = mybir.dt.float32
BF16 = mybir.dt.bfloat16
TANH = mybir.ActivationFunctionType.Tanh

N_CORES = 8
B_FULL = 2048
B_LOC = B_FULL // N_CORES  # 256
NPIX = 784                 # 28*28
XQ_LEN = 1024              # 32*32 padded plane
VROW = 904                 # per-group V row pitch: 28*32 window + dy/dx slack
HALF = 392                 # half pixel plane
NC_MAX = 4                 # groups per stage-B chunk

# image -> (round r, group j, strip s): img = 125*r + 5*j + s
ROUNDS = ((0, 25, 125), (1, 25, 125), (2, 2, 6))  # (r, n_groups, rows stored)


def _emit(ctx, tc, x_d, wblk_d, ones5_d, bias_d, y_d):
    nc = tc.nc

    cpool = ctx.enter_context(tc.tile_pool(name="const", bufs=1))
    apool = ctx.enter_context(tc.tile_pool(name="v900", bufs=4))
    vpool = ctx.enter_context(tc.tile_pool(name="v25", bufs=4))
    gpool = ctx.enter_context(tc.tile_pool(name="g", bufs=4))
    mpool = ctx.enter_context(tc.tile_pool(name="m", bufs=6))
    epool = ctx.enter_context(tc.tile_pool(name="evac", bufs=3))
    pfc = ctx.enter_context(tc.tile_pool(name="pfc", bufs=2, space="PSUM"))
    pred = ctx.enter_context(tc.tile_pool(name="pred", bufs=2, space="PSUM"))

    # consts ride the scalar queue: sync/gpsimd must start gathering ASAP
    wblk = cpool.tile([125, 125], BF16)
    nc.scalar.dma_start(wblk[:], wblk_d[:])
    mbig = cpool.tile([125, 245], BF16)
    nc.scalar.dma_start(mbig[:], ones5_d[:])
    biasv = cpool.tile([125, 1], F32)
    nc.scalar.dma_start(biasv[:], bias_d[:])

    # padded bf16 images: partition p holds xq of image 125*r + p at cols
    # r*1024. 32 extra tail cols: stage-A reads run to 1024*r + 1032.
    xq = cpool.tile([128, 3 * XQ_LEN + 32], BF16)
    stage = cpool.tile([128, 3 * NPIX], F32)
    # per-round memsets so round 0's cast isn't gated on zeroing it all
    nc.vector.memset(xq[:, 0 : XQ_LEN + 32], 0.0)
    nc.vector.memset(xq[:, XQ_LEN + 32 : 2 * XQ_LEN + 32], 0.0)
    nc.vector.memset(xq[:, 2 * XQ_LEN + 32 :], 0.0)
    # bulk loads ride the scalar ring AND are emitted lazily mid-loop so
    # their transfers don't compete with the first chunks' gathers for
    # the shared DMA engines; only round 0's head loads up front.
    def emit_load(r, lo, hi, leng):
        leng.dma_start(
            stage[lo:hi, NPIX * r : NPIX * (r + 1)],
            x_d[125 * r + lo : 125 * r + hi, :],
        )
        src = stage[lo:hi, NPIX * r : NPIX * (r + 1)].rearrange(
            "p (y x) -> p y x", x=28
        )
        dst = xq[lo:hi, XQ_LEN * r : XQ_LEN * (r + 1)].rearrange(
            "p (y x) -> p y x", x=32
        )[:, 2:30, 2:30]
        nc.vector.tensor_copy(dst, src)

    emit_load(0, 0, 64, nc.sync)
    # emitted after the Nth global chunk: (N, args)
    deferred_loads = [
        (1, (0, 64, 125, nc.scalar)),
        (3, (1, 0, 125, nc.scalar)),
        (6, (2, 0, 6, nc.scalar)),
    ]

    xq_ap = xq[:]
    xq_pitch = xq_ap.ap[0][0]  # partition stride in elements

    # gather/output DMAs alternate between sync and gpsimd; scalar is
    # saturated by the tanh ACTIVATEs.
    issuers = [nc.sync, nc.gpsimd]
    chunk_idx = 0

    for r, n_groups, rows in ROUNDS:
        red_a = pred.tile([125, HALF], F32, tag="red_a")
        red_b = pred.tile([125, HALF], F32, tag="red_b")
        # round 0 ramps up chunk size so the first FC starts ASAP
        if r == 0:
            sizes = [1, 2] + [NC_MAX] * ((n_groups - 3) // NC_MAX)
            sizes += [n_groups - sum(sizes)] if sum(sizes) < n_groups else []
        else:
            sizes = [NC_MAX] * (n_groups // NC_MAX)
            sizes += [n_groups - sum(sizes)] if sum(sizes) < n_groups else []
        j0 = 0
        for ncg in sizes:
            # first chunks' B rides gpsimd's 16-engine queue to beat the
            # startup contention on the 5 shared HWDGE engines
            eng = nc.gpsimd if chunk_idx < 4 else issuers[chunk_idx % 2]
            chunk_idx += 1
            while deferred_loads and chunk_idx > deferred_loads[0][0]:
                emit_load(*deferred_loads.pop(0)[1])

            # --- 1a. stage A: V900g [25=(s,dy), ncg*904+8] per group ---
            # the chunk's A-DMAs run on BOTH issuers in parallel
            v900 = apool.tile([25, NC_MAX * VROW + 8], BF16)
            v900_ap = v900[:]
            v900_pitch = v900_ap.ap[0][0]
            for g in range(ncg):
                in_a = bass.AP(
                    tensor=xq_ap.tensor,
                    offset=xq_pitch * 5 * (j0 + g) + XQ_LEN * r,
                    ap=[[xq_pitch, 5], [32, 5], [1, VROW]],
                )
                issuers[g % 2].dma_start(
                    v900[:, VROW * g : VROW * (g + 1)], in_a
                )

            # --- 1b. stage B: V25 [125=(s,dy,dx), ncg*904] in one DMA ---
            # (partition-stride dim must be the first AP dim)
            v = vpool.tile([125, NC_MAX * VROW + 8], BF16)
            in_b = bass.AP(
                tensor=v900_ap.tensor,
                offset=v900_ap.offset,
                ap=[[v900_pitch, 25], [1, 5], [1, VROW * ncg]],
            )
            eng.dma_start(v[:, 0 : VROW * ncg], in_b)

            for g in range(ncg):
                # strided views of the real 28x28 pixel plane
                vyx = (
                    v[:, VROW * g : VROW * g + 896]
                    .rearrange("p (y xc) -> p y xc", xc=32)[:, :, 0:28]
                )

                # --- 2. FC matmuls into one 2-bank PSUM tile ---
                fc = pfc.tile([125, 1024], F32)
                nc.tensor.matmul(
                    fc[:, 0:HALF], wblk[:], vyx[:, 0:14],
                    start=True, stop=True,
                )
                nc.tensor.matmul(
                    fc[:, 512 : 512 + HALF], wblk[:], vyx[:, 14:28],
                    start=True, stop=True,
                )

                # --- 3. G = tanh(FC + b), one strided ACT over both banks ---
                g_t = gpool.tile([125, NPIX], BF16)
                fcv = fc[:].rearrange("p (t c) -> p t c", c=512)[:, :, 0:HALF]
                gv = g_t[:].rearrange("p (t c) -> p t c", c=HALF)
                nc.scalar.activation(gv, fcv, TANH, bias=biasv[:], scale=1.0)

                # --- 4. M = V * G (DVE) ---
                m = mpool.tile([125, NPIX], BF16)
                gyx = g_t[:].rearrange("p (y x) -> p y x", x=28)
                myx = m[:].rearrange("p (y x) -> p y x", x=28)
                nc.vector.tensor_mul(myx, vyx, gyx)

                # --- 5. per-strip channel reduce, placed at partitions 5j.. ---
                j = j0 + g
                ones_j = mbig[:, 120 - 5 * j : 245 - 5 * j]
                nc.tensor.matmul(
                    red_a[:], ones_j, m[:, 0:HALF],
                    start=(j == 0), stop=(j == n_groups - 1),
                    skip_group_check=True,
                )
                nc.tensor.matmul(
                    red_b[:], ones_j, m[:, HALF:NPIX],
                    start=(j == 0), stop=(j == n_groups - 1),
                    skip_group_check=True,
                )
            j0 += ncg

        # --- 6. evacuate + store round ---
        e_a = epool.tile([125, HALF], F32, tag="e_a")
        nc.vector.tensor_copy(e_a[:], red_a[:])
        nc.sync.dma_start(y_d[125 * r : 125 * r + rows, 0:HALF], e_a[0:rows, :])
        e_b = epool.tile([125, HALF], F32, tag="e_b")
        nc.vector.tensor_copy(e_b[:], red_b[:])
        nc.gpsimd.dma_start(
            y_d[125 * r : 125 * r + rows, HALF:NPIX], e_b[0:rows, :]
        )


def build():
    nc = bacc.Bacc("TRN2", target_bir_lowering=False, debug=False)
    x_d = nc.dram_tensor("x", [B_LOC, NPIX], F32, kind="ExternalInput").ap()
    wblk_d = nc.dram_tensor("wblk", [125, 125], BF16, kind="ExternalInput").ap()
    ones5_d = nc.dram_tensor("mbig", [125, 245], BF16, kind="ExternalInput").ap()
    bias_d = nc.dram_tensor("biasv", [125, 1], F32, kind="ExternalInput").ap()
    y_d = nc.dram_tensor("y", [B_LOC, NPIX], F32, kind="ExternalOutput").ap()

    with tile.TileContext(nc) as tc:
        with ExitStack() as ctx:
            _emit(ctx, tc, x_d, wblk_d, ones5_d, bias_d, y_d)
    nc.compile()
    return nc


def make_consts(W, b):
    import ml_dtypes

    W = np.asarray(W, dtype=np.float32)
    b = np.asarray(b, dtype=np.float32)
    # W2[c, 5*dx+dy] = W[c, 0, dy, dx]
    W2 = W[:, 0].transpose(0, 2, 1).reshape(25, 25)
    wblk = np.zeros((125, 125), dtype=np.float32)
    for s in range(5):
        wblk[25 * s : 25 * s + 25, 25 * s : 25 * s + 25] = W2.T
    mbig = np.zeros((125, 245), dtype=np.float32)
    for s in range(5):
        mbig[25 * s : 25 * s + 25, 120 + s] = 1.0
    biasv = np.tile(b, 5).astype(np.float32)[:, None]
    # permute from the (s, k) layout to the gather's q = (s, dy, dx) layout
    perm = np.zeros(125, dtype=np.int64)
    for s in range(5):
        for dy in range(5):
            for dx in range(5):
                perm[25 * s + 5 * dy + dx] = 25 * s + 5 * dx + dy
    wblk = wblk[perm][:, perm]
    mbig = mbig[perm]
    biasv = biasv[perm]
    wblk = wblk.astype(ml_dtypes.bfloat16)
    mbig = mbig.astype(ml_dtypes.bfloat16)
    return wblk, mbig, biasv


_NC_CACHE = None


def get_nc():
    global _NC_CACHE
    if _NC_CACHE is None:
        _NC_CACHE = build()
    return _NC_CACHE


def run(x, W, b, **spmd_kwargs):
    x = np.ascontiguousarray(np.asarray(x, dtype=np.float32))
    wblk, mbig, biasv = make_consts(W, b)
    xs = x.reshape(N_CORES, B_LOC, NPIX)
    in_maps = [
        {"x": xs[c], "wblk": wblk, "mbig": mbig, "biasv": biasv}
        for c in range(N_CORES)
    ]
    nc = get_nc()
    res = bass_utils.run_bass_kernel_spmd(
        nc, in_maps, list(range(N_CORES)), **spmd_kwargs
    )
    y = np.concatenate([res.results[c]["y"] for c in range(N_CORES)], axis=0)
    return y.reshape(B_FULL, 1, 28, 28), res


def kernel(x, W, b):
    y, _ = run(x, W, b)
    return y.astype(np.float32)
